# revision 1
# baseline (speedup 1.0000x reference)
"""Trainium2 Bass kernel for NeuralComplexityLoss (sample-entropy MSE).

Contract: kernel(predictions, targets) -> np.float32 scalar (shape ()),
matching reference.reference(). Self-contained: hardcodes shapes/sharding.

Strategy
--------
128 signals (64 pred + 64 targ, each length T=1024) are sharded 16 per core
across 8 NeuronCores. Host normalizes each signal (mean/ddof-1 std, f64 ->
f32). Each core computes, per signal, the two sample-entropy match counts
over the upper triangle {(i, j): 0 <= i <= j < N}, N = T - M = 1022:

  cnt2 = #{(i,j): max_k<2 |x[i+k]-x[j+k]| <= R}   (incl. diagonal)
  cnt3 = #{(i,j): max_k<3 |x[i+k]-x[j+k]| <= R}

By symmetry the reference's full-matrix counts are 2*cnt - N. Per 126-row
strip the kernel computes a_k = |x[j+k] - x[i+k]| (ACT Abs with per-
partition bias), then chains masked indicator products on DVE with fused
row-sum accumulation:

  i0 = (a0 <= R) * mask          (mask = upper-triangular ones)
  c2 = (a1 <= R) * i0,  accum -> cnt2 row sums
  c3 = (a2 <= R) * c2,  accum -> cnt3 row sums

Per-strip row sums land in distinct columns of a [128, 288] stats tile,
DMA'd out once; host reduces, forms entropies and the final MSE.
"""

import copy

import numpy as np

B, C, T = 4, 16, 1024
M = 2
R = 0.2
EPS = 1e-8
N = T - M                      # 1022 templates
NCORES = 8
NSIG = 2 * B * C               # 128 signals total
S_PER_CORE = NSIG // NCORES    # 16
STRIP = 126                    # valid output rows per strip (128 - M)
NSTRIPS = (N + STRIP - 1) // STRIP   # 9
STATS_COLS = S_PER_CORE * 2 * NSTRIPS  # 288

_CACHE = {}
LAST_RESULTS = None  # stashed BassKernelResults for test.py profiling


def _split_excess_waits(nc, maxw=1):
    """This walrus codegen accepts only one sync-wait per instruction:
    hoist extras onto preceding single-wait NOPs on the same engine."""
    import bass_rust
    import concourse.mybir as mybir

    n_split = 0
    for bb in nc.main_func.blocks:
        insts = bb.instructions
        i = 0
        while i < len(insts):
            ins = insts[i]
            si = ins.sync_info
            waits = list(si.on_wait) if si is not None and si.on_wait else []
            if len(waits) > maxw:
                extra, keep = waits[:-maxw], waits[-maxw:]
                nops = []
                for j, w in enumerate(extra):
                    nop = bass_rust.InstNoOp(
                        name=f"{ins.name}-wsplit{j}", ins=[], outs=[]
                    )
                    nop.engine = ins.engine
                    nop.sync_info = mybir.SyncInfo(on_wait=[w], on_update=[])
                    nops.append(nop)
                si.on_wait = keep
                insts[i:i] = nops
                i += len(nops)
                n_split += 1
            i += 1
    return n_split


def _build():
    import concourse.bass as bass
    import concourse.tile as tile
    from concourse import mybir
    from concourse.alu_op_type import AluOpType

    f32 = mybir.dt.float32
    nc = bass.Bass(trn_type="TRN2", num_devices=NCORES)
    x = nc.dram_tensor("x", [S_PER_CORE, T], f32, kind="ExternalInput")
    out = nc.dram_tensor("cnt", [128, STATS_COLS], f32, kind="ExternalOutput")

    mask_np = np.triu(np.ones((128, N), dtype=np.float32))
    mask_dram = nc.inline_tensor(mask_np, name="trimask")

    xa = x.ap()
    with tile.TileContext(nc) as tc:
        with (
            tc.tile_pool(name="singles", bufs=1) as singles,
            tc.tile_pool(name="reps", bufs=2) as reps,
            tc.tile_pool(name="cols", bufs=2) as cols,
            tc.tile_pool(name="work", bufs=3) as work,
        ):
            mask = singles.tile([128, N], f32)
            nc.sync.dma_start(out=mask, in_=mask_dram[:, :])
            stats = singles.tile([128, STATS_COLS], f32)
            nc.vector.memset(stats, 0.0)

            for s in range(S_PER_CORE):
                row = xa[s : s + 1, :]  # [1, T] AP into DRAM
                x_rep = reps.tile([128, T], f32)
                nc.sync.dma_start(
                    out=x_rep,
                    in_=bass.AP(
                        tensor=row.tensor, offset=row.offset, ap=[[0, 128], [1, T]]
                    ),
                )
                # xcolT[p, 3*rt + k] = x[126*rt + k + p]
                xcolT = cols.tile([128, 3 * NSTRIPS], f32)
                ncolT = cols.tile([128, 3 * NSTRIPS], f32)
                nc.sync.dma_start(
                    out=xcolT[:, 0 : 3 * (NSTRIPS - 1)],
                    in_=bass.AP(
                        tensor=row.tensor,
                        offset=row.offset,
                        ap=[[1, 128], [STRIP, NSTRIPS - 1], [1, 3]],
                    ),
                )
                last_rows = N - STRIP * (NSTRIPS - 1)  # 14
                nc.sync.dma_start(
                    out=xcolT[0:last_rows, 3 * (NSTRIPS - 1) : 3 * NSTRIPS],
                    in_=bass.AP(
                        tensor=row.tensor,
                        offset=row.offset + STRIP * (NSTRIPS - 1),
                        ap=[[1, last_rows], [1, 3]],
                    ),
                )
                nc.vector.tensor_scalar_mul(
                    ncolT[:, 0 : 3 * (NSTRIPS - 1)], xcolT[:, 0 : 3 * (NSTRIPS - 1)], -1.0
                )
                nc.vector.tensor_scalar_mul(
                    ncolT[0:last_rows, 3 * (NSTRIPS - 1) :],
                    xcolT[0:last_rows, 3 * (NSTRIPS - 1) :],
                    -1.0,
                )

                for rt in range(NSTRIPS):
                    r0 = STRIP * rt
                    nrows = min(STRIP, N - r0)
                    wc = N - r0  # count width: j in [r0, N)
                    a0 = work.tile([STRIP, N], f32)
                    a1 = work.tile([STRIP, N], f32)
                    a2 = work.tile([STRIP, N], f32)
                    for k, ak in enumerate((a0, a1, a2)):
                        nc.scalar.activation(
                            out=ak[0:nrows, 0:wc],
                            in_=x_rep[0:nrows, r0 + k : r0 + k + wc],
                            func=mybir.ActivationFunctionType.Abs,
                            bias=ncolT[0:nrows, 3 * rt + k : 3 * rt + k + 1],
                            scale=1.0,
                        )
                    i0 = work.tile([STRIP, N], f32)
                    c2 = work.tile([STRIP, N], f32)
                    c3 = work.tile([STRIP, N], f32)
                    nc.vector.scalar_tensor_tensor(
                        out=i0[0:nrows, 0:wc],
                        in0=a0[0:nrows, 0:wc],
                        scalar=R,
                        in1=mask[0:nrows, 0:wc],
                        op0=AluOpType.is_le,
                        op1=AluOpType.mult,
                    )
                    col2 = s * 2 * NSTRIPS + rt
                    col3 = s * 2 * NSTRIPS + NSTRIPS + rt
                    nc.vector.scalar_tensor_tensor(
                        out=c2[0:nrows, 0:wc],
                        in0=a1[0:nrows, 0:wc],
                        scalar=R,
                        in1=i0[0:nrows, 0:wc],
                        op0=AluOpType.is_le,
                        op1=AluOpType.mult,
                        accum_out=stats[0:nrows, col2 : col2 + 1],
                    )
                    nc.vector.scalar_tensor_tensor(
                        out=c3[0:nrows, 0:wc],
                        in0=a2[0:nrows, 0:wc],
                        scalar=R,
                        in1=c2[0:nrows, 0:wc],
                        op0=AluOpType.is_le,
                        op1=AluOpType.mult,
                        accum_out=stats[0:nrows, col3 : col3 + 1],
                    )

            nc.sync.dma_start(out=out[:, :], in_=stats)

    _split_excess_waits(nc)
    return nc


def _get_nc():
    if "nc" not in _CACHE:
        _CACHE["nc"] = _build()
    return _CACHE["nc"]


def kernel(predictions, targets, _trace=False):
    global LAST_RESULTS
    from concourse.bass_utils import run_bass_kernel_spmd

    preds = np.asarray(predictions, dtype=np.float32).reshape(B * C, T)
    targs = np.asarray(targets, dtype=np.float32).reshape(B * C, T)
    xall = np.concatenate([preds, targs], axis=0)  # [128, T]

    mu = xall.mean(axis=1, dtype=np.float64)
    sd = xall.std(axis=1, ddof=1, dtype=np.float64)
    xhat = ((xall - mu[:, None]) / (sd[:, None] + EPS)).astype(np.float32)

    nc = _get_nc()
    in_maps = [
        {"x": np.ascontiguousarray(xhat[c * S_PER_CORE : (c + 1) * S_PER_CORE])}
        for c in range(NCORES)
    ]
    res = run_bass_kernel_spmd(
        nc, in_maps, core_ids=list(range(NCORES)), trace=_trace
    )
    LAST_RESULTS = res

    ents = np.zeros(NSIG, dtype=np.float64)
    for c in range(NCORES):
        stats = res.results[c]["cnt"].astype(np.float64)  # [128, 288]
        for s in range(S_PER_CORE):
            base = s * 2 * NSTRIPS
            cnt2 = stats[:, base : base + NSTRIPS].sum()
            cnt3 = stats[:, base + NSTRIPS : base + 2 * NSTRIPS].sum()
            m = 2.0 * cnt2 - N
            m1 = 2.0 * cnt3 - N
            ratio = m1 / max(m, 1.0)
            ent = -np.log(max(ratio, 1e-30)) if (m > 0 and m1 > 0) else 0.0
            ents[c * S_PER_CORE + s] = ent

    ep = ents[: B * C].reshape(B, C)
    et = ents[B * C :].reshape(B, C)
    return np.array(np.mean((ep - et) ** 2), dtype=np.float32)
